# revision 6
# baseline (speedup 1.0000x reference)
"""Trainium2 Bass kernel for nn_ColRepeatCausalLinear.

Math: reference computes out = x @ W + bias with
    W[s, t] = v[t] * d^(t-s)  for t >= s, else 0,   d = clip(decay_value, 0.9, 1)
which factorizes as a decayed prefix scan along S:
    y[b, e, t] = d * y[b, e, t-1] + x[b, e, t]
    out[b, e, t] = v[t] * y[b, e, t] + bias[t]
i.e. O(B*E*S) work instead of the O(B*E*S^2) dense matmul.

Mapping: data-parallel over B across 8 NeuronCores (x[b] per core, params
replicated). Per core the kernel sits on the DMA wall: 8 MiB in + 8 MiB
out against ~27 GB/s/engine x 16 SDMA engines (~430 GB/s). Everything is
structured to keep that pipe saturated end to end:
  - x/out move in [128, 2*S] tiles (partition p holds two consecutive
    e-rows, 16 KiB contiguous per descriptor), whole tiles alternating
    between the two HWDGE queues; one queue's ~69 ns/descriptor dispatch
    rate caps at ~256 GB/s, two together reach the engine ceiling.
  - all load triggers enqueue before any store trigger, so a store
    waiting on compute never head-of-line blocks a load.
  - first load and last store are split half/half across both queues:
    the first tile lands (and the Vector engine starts) ~9 us earlier,
    and the final store drains at the two-queue rate.
  - v is known at build time, so a bf16 hi/lo split of it ships as a
    NEFF-embedded constant; a K=2 ones-matmul broadcasts hi+lo across
    partitions into PSUM (exact to ~2^-17) ~2 us after the 8 KiB row
    lands — no 1 MiB broadcast read, no slow fp32 PE pass.
  - the scan+scale runs per e-row chunk on the Vector engine via a fused
    custom DVE op (cumsum * v in one pass) reading v straight from PSUM.

Hardcoded problem shapes: x (8, 1024, 2048) f32, weight (1, 2048),
bias (2048,), decay_value (1,).
"""

import numpy as np
import ml_dtypes

import concourse.bacc as bacc
import concourse.mybir as mybir
from concourse.tile import TileContext
from concourse.bass_utils import run_bass_kernel_spmd

B, E, S = 8, 1024, 2048
P = 128
N_CORES = 8
F32 = mybir.dt.float32

_cache = {}

# Fused custom DVE op: out[p,k] = (sum_{j<=k} x[p,j]) * v[p,k] — the whole
# d=1 kernel body in ONE Vector-engine instruction (the stock path needs a
# 2-cyc/elem TensorTensorScan plus a 1-cyc/elem tensor_mul). Registered at
# runtime into dve_ops.OPS; sha self-pinned since this op isn't in-tree.
_FUSED_OP = None
try:
    from concourse import dve_ops as _dops
    from concourse.dve_spec import AluOp as _AluOp, Spec as _Spec
    from concourse.dve_spec import Src0 as _Src0, Src1 as _Src1, scan as _scan
    from concourse.dve_spec import lower as _lower
    from concourse.dve_uop import DveOpSpec as _DveOpSpec

    _FUSED_NAME = "CUMSUM_VSCALE_ANT"
    if _FUSED_NAME in _dops._SUB_OPCODE_FOR_NAME:
        _FUSED_OP = next(o for o in _dops.OPS if o.name == _FUSED_NAME)
    else:
        _fspec = _Spec(body=_scan(_AluOp.ADD, _Src0) * _Src1)
        _row = _dops._CUSTOM_DVE_ROW_BASE + len(_dops.OPS)
        assert _row < 0x20
        _dops._SUB_OPCODE_FOR_NAME[_FUSED_NAME] = _row
        _sha = {}
        for _ver in ("v3", "v4"):
            try:
                _sha[_ver] = _DveOpSpec(
                    name=_FUSED_NAME,
                    opcode=_row,
                    uops=_lower(_fspec, ver=_ver),
                    rd1_en=_dops.has_src1(_fspec),
                ).sha(_ver)
            except Exception:
                pass
        _FUSED_OP = _dops.DveOp(_FUSED_NAME, _fspec, subdim=False, uops_sha=_sha)
        _dops.OPS.append(_FUSED_OP)
        _dops.CUSTOM_DVE_SPECS[_FUSED_NAME] = _fspec
except Exception:
    _FUSED_OP = None

R = 2  # e-rows per partition per DMA transfer (16 KiB descriptors)
BANK = 512  # fp32 elems per PSUM bank


def _bf16_hilo(row: np.ndarray) -> np.ndarray:
    """[S] f32 -> [2, S] bf16 with hi + lo == row to ~2^-17 relative."""
    hi = row.astype(ml_dtypes.bfloat16)
    lo = (row - hi.astype(np.float32)).astype(ml_dtypes.bfloat16)
    return np.ascontiguousarray(np.stack([hi, lo]))


def _build(d: float, has_bias: bool, v_np: np.ndarray, bias_np: np.ndarray):
    nc = bacc.Bacc(
        "TRN2",
        target_bir_lowering=False,
        debug=False,
        enable_asserts=False,
    )
    x = nc.dram_tensor("x", [E, S], F32, kind="ExternalInput").ap()
    out = nc.dram_tensor("out", [E, S], F32, kind="ExternalOutput").ap()
    hilo_dram = nc.inline_tensor(_bf16_hilo(v_np), name="vhilo").ap()
    if has_bias:
        bhilo_dram = nc.inline_tensor(_bf16_hilo(bias_np), name="bhilo").ap()

    n_tiles = E // (P * R)
    rows = P * R
    H = P // 2
    BF16 = mybir.dt.bfloat16

    with TileContext(nc) as tc:
        with (
            tc.tile_pool(name="const", bufs=1) as cpool,
            tc.tile_pool(name="xs", bufs=n_tiles) as xpool,
            tc.tile_pool(name="ys", bufs=2) as ypool,
            tc.tile_pool(name="os", bufs=n_tiles) as opool,
            tc.tile_pool(name="ps", bufs=1, space="PSUM") as ppool,
        ):
            hilo = cpool.tile([2, S], BF16)
            nc.sync.dma_start(out=hilo[:], in_=hilo_dram)
            ones = cpool.tile([2, P], BF16)
            nc.vector.memset(ones[:], 1.0)
            if has_bias:
                bhilo = cpool.tile([2, S], BF16)
                nc.scalar.dma_start(out=bhilo[:], in_=bhilo_dram)

            # Load schedule: first/last tiles split across both queues,
            # middle tiles whole — queue desc counts stay balanced while the
            # first tile lands in half the time.
            xts = []
            for i in range(n_tiles):
                xt = xpool.tile([P, R * S], F32)
                src = x[i * rows : (i + 1) * rows, :].rearrange(
                    "(p b) s -> p (b s)", b=R
                )
                if i == 0 or i == n_tiles - 1:
                    nc.sync.dma_start(out=xt[:H], in_=src[:H])
                    nc.scalar.dma_start(out=xt[H:], in_=src[H:])
                else:
                    eng = nc.scalar if i % 2 == 1 else nc.sync
                    eng.dma_start(out=xt[:], in_=src)
                xts.append(xt)

            # Broadcast v across partitions: PSUM[p, t] = 1*hi[t] + 1*lo[t].
            vb = ppool.tile([P, S], F32)
            for n in range(S // BANK):
                nc.tensor.matmul(
                    vb[:, n * BANK : (n + 1) * BANK],
                    ones[:],
                    hilo[:, n * BANK : (n + 1) * BANK],
                    start=True,
                    stop=True,
                )
            if has_bias:
                bb = ppool.tile([P, S], F32)
                for n in range(S // BANK):
                    nc.tensor.matmul(
                        bb[:, n * BANK : (n + 1) * BANK],
                        ones[:],
                        bhilo[:, n * BANK : (n + 1) * BANK],
                        start=True,
                        stop=True,
                    )
            if not (d == 1.0 and _FUSED_OP is not None):
                dtile = cpool.tile([P, 1], F32)
                nc.gpsimd.memset(dtile[:], d)

            for i in range(n_tiles):
                xt = xts[i]
                ot = opool.tile([P, R * S], F32)
                for c in range(R):
                    xc = xt[:, c * S : (c + 1) * S]
                    oc = ot[:, c * S : (c + 1) * S]
                    if d == 1.0 and _FUSED_OP is not None:
                        nc.vector._custom_dve(_FUSED_OP, out=oc, in0=xc, in1=vb[:])
                    else:
                        yt = ypool.tile([P, S], F32)
                        nc.vector.tensor_tensor_scan(
                            yt[:], dtile[:].broadcast_to([P, S]), xc,
                            0.0, mybir.AluOpType.mult, mybir.AluOpType.add,
                        )
                        nc.vector.tensor_mul(oc, yt[:], vb[:])
                    if has_bias:
                        nc.vector.tensor_add(oc, oc, bb[:])
                dst = out[i * rows : (i + 1) * rows, :].rearrange(
                    "(p b) s -> p (b s)", b=R
                )
                if i == 0 or i == n_tiles - 1:
                    nc.sync.dma_start(out=dst[:H], in_=ot[:H])
                    nc.scalar.dma_start(out=dst[H:], in_=ot[H:])
                else:
                    # opposite queue from this tile's load
                    eng = nc.sync if i % 2 == 1 else nc.scalar
                    eng.dma_start(out=dst, in_=ot[:])
    nc.compile()
    return nc


def _run(x, weight, bias, decay_value, trace=False):
    x = np.asarray(x, dtype=np.float32)
    weight = np.asarray(weight, dtype=np.float32)
    bias = np.asarray(bias, dtype=np.float32)
    decay_value = np.asarray(decay_value)
    assert x.shape == (B, E, S), x.shape

    # DECAY_CONSTANT = 1.0 in the reference; exponent is (t - s) / 1.0.
    d = float(np.clip(np.float64(decay_value.reshape(-1)[0]), 0.9, 1.0))
    has_bias = bool(np.any(bias))

    v_np = weight.reshape(S).astype(np.float32)
    b_np = bias.reshape(S).astype(np.float32)
    key = (d, has_bias, v_np.tobytes(), b_np.tobytes() if has_bias else b"")
    if key not in _cache:
        _cache[key] = _build(d, has_bias, v_np, b_np)
    nc = _cache[key]

    in_maps = [{"x": np.ascontiguousarray(x[b])} for b in range(N_CORES)]

    res = run_bass_kernel_spmd(
        nc, in_maps, core_ids=list(range(N_CORES)), trace=trace
    )
    out = np.stack([r["out"] for r in res.results], axis=0)
    return out, res


def kernel(x, weight, bias, decay_value):
    out, _ = _run(x, weight, bias, decay_value)
    return out


# revision 8
# speedup vs baseline: 1.1764x; 1.1764x over previous
"""Trainium2 Bass kernel for nn_ColRepeatCausalLinear.

Math: reference computes out = x @ W + bias with
    W[s, t] = v[t] * d^(t-s)  for t >= s, else 0,   d = clip(decay_value, 0.9, 1)
which factorizes as a decayed prefix scan along S:
    y[b, e, t] = d * y[b, e, t-1] + x[b, e, t]
    out[b, e, t] = v[t] * y[b, e, t] + bias[t]
i.e. O(B*E*S) work instead of the O(B*E*S^2) dense matmul.

Mapping: data-parallel over B across 8 NeuronCores (x[b] per core, params
replicated). Per core the kernel sits on the DMA wall: 8 MiB in + 8 MiB
out against ~27 GB/s/engine x 16 SDMA engines (~430 GB/s). Everything is
structured to keep that pipe saturated end to end:
  - x/out move in [128, 2*S] tiles (partition p holds two consecutive
    e-rows, 16 KiB contiguous per descriptor), whole tiles alternating
    between the two HWDGE queues; one queue's ~69 ns/descriptor dispatch
    rate caps at ~256 GB/s, two together reach the engine ceiling.
  - all load triggers enqueue before any store trigger, so a store
    waiting on compute never head-of-line blocks a load.
  - first load and last store are split half/half across both queues:
    the first tile lands (and the Vector engine starts) ~9 us earlier,
    and the final store drains at the two-queue rate.
  - v is known at build time, so a bf16 hi/lo split of it ships as a
    NEFF-embedded constant; a K=2 ones-matmul broadcasts hi+lo across
    partitions into PSUM (exact to ~2^-17) ~2 us after the 8 KiB row
    lands — no 1 MiB broadcast read, no slow fp32 PE pass.
  - the scan+scale runs per e-row chunk on the Vector engine via a fused
    custom DVE op (cumsum * v in one pass) reading v straight from PSUM.

Hardcoded problem shapes: x (8, 1024, 2048) f32, weight (1, 2048),
bias (2048,), decay_value (1,).
"""

import numpy as np
import ml_dtypes

import concourse.bacc as bacc
import concourse.mybir as mybir
from concourse.tile import TileContext
from concourse.bass_utils import run_bass_kernel_spmd

B, E, S = 8, 1024, 2048
P = 128
N_CORES = 8
F32 = mybir.dt.float32

_cache = {}

# Fused custom DVE op: out[p,k] = (sum_{j<=k} x[p,j]) * v[p,k] — the whole
# d=1 kernel body in ONE Vector-engine instruction (the stock path needs a
# 2-cyc/elem TensorTensorScan plus a 1-cyc/elem tensor_mul). Registered at
# runtime into dve_ops.OPS; sha self-pinned since this op isn't in-tree.
_FUSED_OP = None
try:
    from concourse import dve_ops as _dops
    from concourse.dve_spec import AluOp as _AluOp, Spec as _Spec
    from concourse.dve_spec import Src0 as _Src0, Src1 as _Src1, scan as _scan
    from concourse.dve_spec import lower as _lower
    from concourse.dve_uop import DveOpSpec as _DveOpSpec

    _FUSED_NAME = "CUMSUM_VSCALE_ANT"
    if _FUSED_NAME in _dops._SUB_OPCODE_FOR_NAME:
        _FUSED_OP = next(o for o in _dops.OPS if o.name == _FUSED_NAME)
    else:
        _fspec = _Spec(body=_scan(_AluOp.ADD, _Src0) * _Src1)
        _row = _dops._CUSTOM_DVE_ROW_BASE + len(_dops.OPS)
        assert _row < 0x20
        _dops._SUB_OPCODE_FOR_NAME[_FUSED_NAME] = _row
        _sha = {}
        for _ver in ("v3", "v4"):
            try:
                _sha[_ver] = _DveOpSpec(
                    name=_FUSED_NAME,
                    opcode=_row,
                    uops=_lower(_fspec, ver=_ver),
                    rd1_en=_dops.has_src1(_fspec),
                ).sha(_ver)
            except Exception:
                pass
        _FUSED_OP = _dops.DveOp(_FUSED_NAME, _fspec, subdim=False, uops_sha=_sha)
        _dops.OPS.append(_FUSED_OP)
        _dops.CUSTOM_DVE_SPECS[_FUSED_NAME] = _fspec
except Exception:
    _FUSED_OP = None

R = 2  # e-rows per partition per DMA transfer (16 KiB descriptors)
BANK = 512  # fp32 elems per PSUM bank


def _bf16_hilo(row: np.ndarray) -> np.ndarray:
    """[S] f32 -> [2, S] bf16 with hi + lo == row to ~2^-17 relative."""
    hi = row.astype(ml_dtypes.bfloat16)
    lo = (row - hi.astype(np.float32)).astype(ml_dtypes.bfloat16)
    return np.ascontiguousarray(np.stack([hi, lo]))


def _build(d: float, has_bias: bool, v_np: np.ndarray, bias_np: np.ndarray):
    nc = bacc.Bacc(
        "TRN2",
        target_bir_lowering=False,
        debug=False,
        enable_asserts=False,
    )
    x = nc.dram_tensor("x", [E, S], F32, kind="ExternalInput").ap()
    out = nc.dram_tensor("out", [E, S], F32, kind="ExternalOutput").ap()
    hilo_dram = nc.inline_tensor(_bf16_hilo(v_np), name="vhilo").ap()
    if has_bias:
        bhilo_dram = nc.inline_tensor(_bf16_hilo(bias_np), name="bhilo").ap()

    n_tiles = E // (P * R)
    rows = P * R
    H = P // 2
    BF16 = mybir.dt.bfloat16

    with TileContext(nc) as tc:
        with (
            tc.tile_pool(name="const", bufs=1) as cpool,
            tc.tile_pool(name="xs", bufs=n_tiles) as xpool,
            tc.tile_pool(name="ys", bufs=2) as ypool,
            tc.tile_pool(name="os", bufs=n_tiles) as opool,
            tc.tile_pool(name="ps", bufs=1, space="PSUM") as ppool,
        ):
            hilo = cpool.tile([2, S], BF16)
            nc.sync.dma_start(out=hilo[:], in_=hilo_dram)
            ones = cpool.tile([2, P], BF16)
            nc.vector.memset(ones[:], 1.0)
            if has_bias:
                bhilo = cpool.tile([2, S], BF16)
                nc.scalar.dma_start(out=bhilo[:], in_=bhilo_dram)

            # Load schedule: first/last tiles split across both queues,
            # middle tiles whole — queue desc counts stay balanced while the
            # first tile lands in half the time.
            xts = []
            for i in range(n_tiles):
                xt = xpool.tile([P, R * S], F32)
                src = x[i * rows : (i + 1) * rows, :].rearrange(
                    "(p b) s -> p (b s)", b=R
                )
                (nc.sync if i % 2 == 0 else nc.scalar).dma_start(out=xt[:], in_=src)
                xts.append(xt)

            # Broadcast v across partitions: PSUM[p, t] = 1*hi[t] + 1*lo[t].
            vb = ppool.tile([P, S], F32)
            for n in range(S // BANK):
                nc.tensor.matmul(
                    vb[:, n * BANK : (n + 1) * BANK],
                    ones[:],
                    hilo[:, n * BANK : (n + 1) * BANK],
                    start=True,
                    stop=True,
                )
            if has_bias:
                bb = ppool.tile([P, S], F32)
                for n in range(S // BANK):
                    nc.tensor.matmul(
                        bb[:, n * BANK : (n + 1) * BANK],
                        ones[:],
                        bhilo[:, n * BANK : (n + 1) * BANK],
                        start=True,
                        stop=True,
                    )
            if not (d == 1.0 and _FUSED_OP is not None):
                dtile = cpool.tile([P, 1], F32)
                nc.gpsimd.memset(dtile[:], d)

            for i in range(n_tiles):
                xt = xts[i]
                ot = opool.tile([P, R * S], F32)
                for c in range(R):
                    xc = xt[:, c * S : (c + 1) * S]
                    oc = ot[:, c * S : (c + 1) * S]
                    if d == 1.0 and _FUSED_OP is not None:
                        nc.vector._custom_dve(_FUSED_OP, out=oc, in0=xc, in1=vb[:])
                    else:
                        yt = ypool.tile([P, S], F32)
                        nc.vector.tensor_tensor_scan(
                            yt[:], dtile[:].broadcast_to([P, S]), xc,
                            0.0, mybir.AluOpType.mult, mybir.AluOpType.add,
                        )
                        nc.vector.tensor_mul(oc, yt[:], vb[:])
                    if has_bias:
                        nc.vector.tensor_add(oc, oc, bb[:])
                dst = out[i * rows : (i + 1) * rows, :].rearrange(
                    "(p b) s -> p (b s)", b=R
                )
                # opposite queue from this tile's load
                (nc.scalar if i % 2 == 0 else nc.sync).dma_start(out=dst, in_=ot[:])
    nc.compile()
    return nc


def _run(x, weight, bias, decay_value, trace=False):
    x = np.asarray(x, dtype=np.float32)
    weight = np.asarray(weight, dtype=np.float32)
    bias = np.asarray(bias, dtype=np.float32)
    decay_value = np.asarray(decay_value)
    assert x.shape == (B, E, S), x.shape

    # DECAY_CONSTANT = 1.0 in the reference; exponent is (t - s) / 1.0.
    d = float(np.clip(np.float64(decay_value.reshape(-1)[0]), 0.9, 1.0))
    has_bias = bool(np.any(bias))

    v_np = weight.reshape(S).astype(np.float32)
    b_np = bias.reshape(S).astype(np.float32)
    key = (d, has_bias, v_np.tobytes(), b_np.tobytes() if has_bias else b"")
    if key not in _cache:
        _cache[key] = _build(d, has_bias, v_np, b_np)
    nc = _cache[key]

    in_maps = [{"x": np.ascontiguousarray(x[b])} for b in range(N_CORES)]

    res = run_bass_kernel_spmd(
        nc, in_maps, core_ids=list(range(N_CORES)), trace=trace
    )
    out = np.stack([r["out"] for r in res.results], axis=0)
    return out, res


def kernel(x, weight, bias, decay_value):
    out, _ = _run(x, weight, bias, decay_value)
    return out


# revision 9
# speedup vs baseline: 1.1850x; 1.0073x over previous
"""Trainium2 Bass kernel for nn_ColRepeatCausalLinear.

Math: reference computes out = x @ W + bias with
    W[s, t] = v[t] * d^(t-s)  for t >= s, else 0,   d = clip(decay_value, 0.9, 1)
which factorizes as a decayed prefix scan along S:
    y[b, e, t] = d * y[b, e, t-1] + x[b, e, t]
    out[b, e, t] = v[t] * y[b, e, t] + bias[t]
i.e. O(B*E*S) work instead of the O(B*E*S^2) dense matmul.

Mapping: data-parallel over B across 8 NeuronCores (x[b] per core, params
replicated). Per core the kernel sits on the DMA wall: 8 MiB in + 8 MiB
out against ~27 GB/s/engine x 16 SDMA engines (~430 GB/s). Everything is
structured to keep that pipe saturated end to end:
  - x/out move in [128, 2*S] tiles (partition p holds two consecutive
    e-rows, 16 KiB contiguous per descriptor), whole tiles alternating
    between the two HWDGE queues; one queue's ~69 ns/descriptor dispatch
    rate caps at ~256 GB/s, two together reach the engine ceiling.
  - all load triggers enqueue before any store trigger, so a store
    waiting on compute never head-of-line blocks a load.
  - first load and last store are split half/half across both queues:
    the first tile lands (and the Vector engine starts) ~9 us earlier,
    and the final store drains at the two-queue rate.
  - v is known at build time, so a bf16 hi/lo split of it ships as a
    NEFF-embedded constant; a K=2 ones-matmul broadcasts hi+lo across
    partitions into PSUM (exact to ~2^-17) ~2 us after the 8 KiB row
    lands — no 1 MiB broadcast read, no slow fp32 PE pass.
  - the scan+scale runs per e-row chunk on the Vector engine via a fused
    custom DVE op (cumsum * v in one pass) reading v straight from PSUM.

Hardcoded problem shapes: x (8, 1024, 2048) f32, weight (1, 2048),
bias (2048,), decay_value (1,).
"""

import numpy as np
import ml_dtypes

import concourse.bacc as bacc
import concourse.mybir as mybir
from concourse.tile import TileContext
from concourse.bass_utils import run_bass_kernel_spmd

B, E, S = 8, 1024, 2048
P = 128
N_CORES = 8
F32 = mybir.dt.float32

_cache = {}

# Fused custom DVE op: out[p,k] = (sum_{j<=k} x[p,j]) * v[p,k] — the whole
# d=1 kernel body in ONE Vector-engine instruction (the stock path needs a
# 2-cyc/elem TensorTensorScan plus a 1-cyc/elem tensor_mul). Registered at
# runtime into dve_ops.OPS; sha self-pinned since this op isn't in-tree.
_FUSED_OP = None
try:
    from concourse import dve_ops as _dops
    from concourse.dve_spec import AluOp as _AluOp, Spec as _Spec
    from concourse.dve_spec import Src0 as _Src0, Src1 as _Src1, scan as _scan
    from concourse.dve_spec import lower as _lower
    from concourse.dve_uop import DveOpSpec as _DveOpSpec

    _FUSED_NAME = "CUMSUM_VSCALE_ANT"
    if _FUSED_NAME in _dops._SUB_OPCODE_FOR_NAME:
        _FUSED_OP = next(o for o in _dops.OPS if o.name == _FUSED_NAME)
    else:
        _fspec = _Spec(body=_scan(_AluOp.ADD, _Src0) * _Src1)
        _row = _dops._CUSTOM_DVE_ROW_BASE + len(_dops.OPS)
        assert _row < 0x20
        _dops._SUB_OPCODE_FOR_NAME[_FUSED_NAME] = _row
        _sha = {}
        for _ver in ("v3", "v4"):
            try:
                _sha[_ver] = _DveOpSpec(
                    name=_FUSED_NAME,
                    opcode=_row,
                    uops=_lower(_fspec, ver=_ver),
                    rd1_en=_dops.has_src1(_fspec),
                ).sha(_ver)
            except Exception:
                pass
        _FUSED_OP = _dops.DveOp(_FUSED_NAME, _fspec, subdim=False, uops_sha=_sha)
        _dops.OPS.append(_FUSED_OP)
        _dops.CUSTOM_DVE_SPECS[_FUSED_NAME] = _fspec
except Exception:
    _FUSED_OP = None

R = 2  # e-rows per partition per DMA transfer (16 KiB descriptors)
BANK = 512  # fp32 elems per PSUM bank


def _bf16_hilo(row: np.ndarray) -> np.ndarray:
    """[S] f32 -> [2, S] bf16 with hi + lo == row to ~2^-17 relative."""
    hi = row.astype(ml_dtypes.bfloat16)
    lo = (row - hi.astype(np.float32)).astype(ml_dtypes.bfloat16)
    return np.ascontiguousarray(np.stack([hi, lo]))


def _build(d: float, has_bias: bool, v_np: np.ndarray, bias_np: np.ndarray):
    nc = bacc.Bacc(
        "TRN2",
        target_bir_lowering=False,
        debug=False,
        enable_asserts=False,
    )
    x = nc.dram_tensor("x", [E, S], F32, kind="ExternalInput").ap()
    out = nc.dram_tensor("out", [E, S], F32, kind="ExternalOutput").ap()
    hilo_dram = nc.inline_tensor(_bf16_hilo(v_np), name="vhilo").ap()
    if has_bias:
        bhilo_dram = nc.inline_tensor(_bf16_hilo(bias_np), name="bhilo").ap()

    n_tiles = E // (P * R)
    rows = P * R
    H = P // 2
    BF16 = mybir.dt.bfloat16

    with TileContext(nc) as tc:
        with (
            tc.tile_pool(name="const", bufs=1) as cpool,
            tc.tile_pool(name="xs", bufs=n_tiles) as xpool,
            tc.tile_pool(name="ys", bufs=2) as ypool,
            tc.tile_pool(name="os", bufs=n_tiles) as opool,
            tc.tile_pool(name="ps", bufs=1, space="PSUM") as ppool,
        ):
            # hilo rides the (otherwise idle) gpsimd SWDGE queue so the two
            # HWDGE queues carry nothing but the eight 2 MiB transfers.
            hilo = cpool.tile([2, S], BF16)
            nc.gpsimd.dma_start(out=hilo[:], in_=hilo_dram)
            ones = cpool.tile([2, P], BF16)
            nc.vector.memset(ones[:], 1.0)
            if has_bias:
                bhilo = cpool.tile([2, S], BF16)
                nc.gpsimd.dma_start(out=bhilo[:], in_=bhilo_dram)

            # Load schedule: first/last tiles split across both queues,
            # middle tiles whole — queue desc counts stay balanced while the
            # first tile lands in half the time.
            xts = []
            for i in range(n_tiles):
                xt = xpool.tile([P, R * S], F32)
                src = x[i * rows : (i + 1) * rows, :].rearrange(
                    "(p b) s -> p (b s)", b=R
                )
                (nc.sync if i % 2 == 0 else nc.scalar).dma_start(out=xt[:], in_=src)
                xts.append(xt)

            # Broadcast v across partitions: PSUM[p, t] = 1*hi[t] + 1*lo[t].
            vb = ppool.tile([P, S], F32)
            for n in range(S // BANK):
                nc.tensor.matmul(
                    vb[:, n * BANK : (n + 1) * BANK],
                    ones[:],
                    hilo[:, n * BANK : (n + 1) * BANK],
                    start=True,
                    stop=True,
                )
            if has_bias:
                bb = ppool.tile([P, S], F32)
                for n in range(S // BANK):
                    nc.tensor.matmul(
                        bb[:, n * BANK : (n + 1) * BANK],
                        ones[:],
                        bhilo[:, n * BANK : (n + 1) * BANK],
                        start=True,
                        stop=True,
                    )
            if not (d == 1.0 and _FUSED_OP is not None):
                dtile = cpool.tile([P, 1], F32)
                nc.gpsimd.memset(dtile[:], d)

            for i in range(n_tiles):
                xt = xts[i]
                ot = opool.tile([P, R * S], F32)
                for c in range(R):
                    xc = xt[:, c * S : (c + 1) * S]
                    oc = ot[:, c * S : (c + 1) * S]
                    if d == 1.0 and _FUSED_OP is not None:
                        nc.vector._custom_dve(_FUSED_OP, out=oc, in0=xc, in1=vb[:])
                    else:
                        yt = ypool.tile([P, S], F32)
                        nc.vector.tensor_tensor_scan(
                            yt[:], dtile[:].broadcast_to([P, S]), xc,
                            0.0, mybir.AluOpType.mult, mybir.AluOpType.add,
                        )
                        nc.vector.tensor_mul(oc, yt[:], vb[:])
                    if has_bias:
                        nc.vector.tensor_add(oc, oc, bb[:])
                dst = out[i * rows : (i + 1) * rows, :].rearrange(
                    "(p b) s -> p (b s)", b=R
                )
                # opposite queue from this tile's load
                (nc.scalar if i % 2 == 0 else nc.sync).dma_start(out=dst, in_=ot[:])
    nc.compile()
    return nc


def _run(x, weight, bias, decay_value, trace=False):
    x = np.asarray(x, dtype=np.float32)
    weight = np.asarray(weight, dtype=np.float32)
    bias = np.asarray(bias, dtype=np.float32)
    decay_value = np.asarray(decay_value)
    assert x.shape == (B, E, S), x.shape

    # DECAY_CONSTANT = 1.0 in the reference; exponent is (t - s) / 1.0.
    d = float(np.clip(np.float64(decay_value.reshape(-1)[0]), 0.9, 1.0))
    has_bias = bool(np.any(bias))

    v_np = weight.reshape(S).astype(np.float32)
    b_np = bias.reshape(S).astype(np.float32)
    key = (d, has_bias, v_np.tobytes(), b_np.tobytes() if has_bias else b"")
    if key not in _cache:
        _cache[key] = _build(d, has_bias, v_np, b_np)
    nc = _cache[key]

    in_maps = [{"x": np.ascontiguousarray(x[b])} for b in range(N_CORES)]

    res = run_bass_kernel_spmd(
        nc, in_maps, core_ids=list(range(N_CORES)), trace=trace
    )
    out = np.stack([r["out"] for r in res.results], axis=0)
    return out, res


def kernel(x, weight, bias, decay_value):
    out, _ = _run(x, weight, bias, decay_value)
    return out
